# revision 50
# baseline (speedup 1.0000x reference)
"""Causal attention kernel for Trainium2, 8 NeuronCores.

Problem: x[4,4096,768] f32; Wq/Wk/Wv [768,64] f32.
  q,k,v = x@W*; S = q@k.T (causal); out = softmax(S/8)@v  -> [4,4096,64] f32.

Sharding: data-parallel over batch (4) x query-range split (2).
  Cores 0-3 run program A (batches 0-3, q rows [0,SPLIT), keys [0,SPLIT)),
  cores 4-7 run program B (batches 0-3, q rows [SPLIT,4096), keys full).

v3: all data paths bf16 (per-element relative noise in q/k/v/P passes
straight to the output rms, so fp8 anywhere fails the 2e-2 gate).
  - projections bf16; kT/qT in kq rows 0:64, vT rows 64:128; v
    DMA-transposed to token-major vx tiles [128,65] with a ones column
    (row 64 of PV = softmax denominators).
  - scores S^T per 128-key tile (pair-groups of 2 for exp batching);
    diagonal tiles masked exactly by a bf16 triangle multiply on DVE or
    Pool (balancer).
  - P = exp(S/8): ACT native exp (bf16 out), or a one-instruction DVE
    Schraudolph: i16 = S*2^7/(8 ln2) + CB16, whose bytes ARE bf16 P;
    the PV matmul reads the i16 tile bitcast as bf16 (~2.2% sawtooth,
    share-limited by the balancer's natural ~30% split).
  - PV bf16 per tile, accumulated in PSUM f32 [65, 512].
  - output: unnormalized oext [65, NQ] f32; host divides + transposes.
"""

import numpy as np
import ml_dtypes

import concourse.bass as bass
import concourse.bacc as bacc
import concourse.mybir as mybir
import concourse.tile as tile
from concourse.bass_utils import run_bass_kernel_spmd

B, N, D_IN, D_OUT = 4, 4096, 768, 64
SPLIT = 2944  # q-row split; multiple of 128
BF16 = mybir.dt.bfloat16
F32 = mybir.dt.float32
I16 = mybir.dt.int16
NPBF16 = ml_dtypes.bfloat16
SCALE = 0.125  # 1/sqrt(64)
# one-instruction Schraudolph: i16 = s*C1 + CB16; the i16 bytes read as
# bf16 give ~exp(s/8).  C1 = 2^7/(8 ln2); CB16 centers the 2^frac~1+frac
# error (* 2^-log2(1.0305)) and folds truncation into rounding (+0.5).
C1 = float(2.0**7 / (8.0 * np.log(2.0)))
CB16 = float(0x3F80 - np.log2(1.0305) * 2.0**7 + 0.5)

# engine cost constants (ns/col, ns/inst) for the greedy balancer
ACT_COL, ACT_INST = 0.833, 185.0
DVE_COL, DVE_COL4, DVE_INST = 1.042, 0.26, 110.0
POOL_MASK = 355.0  # bf16 128-col triangle mult on Pool (eff 0.42) + launch
DVE_MASK = 130.0


def _chunks_for(q0, nq):
    out = []
    c0 = q0
    while c0 < q0 + nq:
        out.append((c0, min(512, q0 + nq - c0)))
        c0 += 512
    return out


class Balancer:
    """Greedy per-engine busy-time tracker."""

    def __init__(self):
        self.t = {"act": 0.0, "dve": 0.0, "pool": 0.0}

    def advance(self, now):
        for e in self.t:
            self.t[e] = max(self.t[e], now)

    def pick_exp(self, cols):
        ca = self.t["act"] + ACT_COL * cols + ACT_INST
        cd = self.t["dve"] + DVE_COL * cols + DVE_INST + 120.0
        if ca <= cd:
            self.t["act"] = ca
            return "act"
        self.t["dve"] = cd
        return "dve"

    def pick_copy(self, cols):
        ca = self.t["act"] + ACT_COL * cols + ACT_INST
        cd = self.t["dve"] + DVE_COL * cols + DVE_INST
        if ca < cd:
            self.t["act"] = ca
            return "act"
        self.t["dve"] = cd
        return "dve"

    def pick_mask(self):
        cd = self.t["dve"] + DVE_MASK
        cp = self.t["pool"] + POOL_MASK
        if cp <= cd:
            self.t["pool"] = cp
            return "pool"
        self.t["dve"] = cd
        return "dve"


def build_half(NK, Q0, NQ, b_mode=False):
    """Build one program. A: NK=SPLIT, Q0=0. B: NK=N, Q0=SPLIT, b_mode."""
    nc = bacc.Bacc("TRN2", target_bir_lowering=False, debug=False)

    nkt = NK // 128
    KQW = NK + NQ

    xT_d = nc.dram_tensor("xT", [D_IN, NK], BF16, kind="ExternalInput")
    w_d = nc.dram_tensor("wqkv", [D_IN, 192], BF16, kind="ExternalInput")
    cst_d = nc.dram_tensor("cst", [128, 128], BF16, kind="ExternalInput")
    oext_d = nc.dram_tensor("oext", [65, NQ], F32, kind="ExternalOutput")

    bal = Balancer()
    from contextlib import ExitStack

    with tile.TileContext(nc) as tc, ExitStack() as stk:
        cpool = stk.enter_context(tc.tile_pool(name="const", bufs=1))
        xpool = stk.enter_context(tc.tile_pool(name="xt", bufs=1))
        jpool = stk.enter_context(tc.tile_pool(name="proj", bufs=1))
        ppool = stk.enter_context(tc.tile_pool(name="pp", bufs=6))
        ipool = stk.enter_context(tc.tile_pool(name="ichain", bufs=6))
        fpool = stk.enter_context(tc.tile_pool(name="fin", bufs=2))
        prpsum = stk.enter_context(tc.tile_pool(name="prpsum", bufs=1, space="PSUM"))
        spsum = stk.enter_context(tc.tile_pool(name="spsum", bufs=3, space="PSUM"))
        opsum = stk.enter_context(tc.tile_pool(name="opsum", bufs=1, space="PSUM"))

        # ---- tiles ----
        w_sb = cpool.tile([128, 6 * 192], BF16, tag="w")
        w3 = w_sb.rearrange("p (c j) -> p c j", j=192)
        zbias = cpool.tile([128, 1], F32, tag="zbias")
        nc.vector.memset(zbias[:, :], 0.0)
        mask_sb = cpool.tile([128, 128], BF16, tag="mask")

        # kq bf16: rows 0:64 = kT (cols 0:NK) and qT (cols NK:), rows
        # 64:128 = vT (cols 0:NK)
        kq_sb = jpool.tile([128, KQW], BF16, tag="kq")

        # v token-major: transpose target + ones-extended tiles
        vn_sb = jpool.tile([128, nkt * 64], BF16, tag="vnat")
        vn3 = vn_sb.rearrange("p (t e) -> p t e", e=64)
        vx_sb = jpool.tile([128, nkt * 65], BF16, tag="vext")
        vx3 = vx_sb.rearrange("p (t e) -> p t e", e=65)
        nc.gpsimd.memset(vx_sb[:, :], 1.0)  # ones cols survive v copies

        xt_sb = xpool.tile([128, 6 * NK], BF16, tag="xt")
        xt3 = xt_sb.rearrange("p (c n) -> p c n", n=NK)
        xT3d = xT_d.ap().rearrange("(c p) n -> p c n", p=128)

        def _ranges(lo, hi):
            out = []
            g0 = lo
            while hi - g0 >= 1024:
                out.append((g0, g0 + 512))
                g0 += 512
            out.append((g0, hi))
            return out

        if b_mode:
            dma_ranges = _ranges(Q0, NK) + _ranges(0, Q0)
        else:
            dma_ranges = _ranges(0, NK)

        # DMA order: weights, first x range, small consts, rest of x
        nc.sync.dma_start(w3, w_d.ap().rearrange("(c p) j -> p c j", p=128))
        r0, r1 = dma_ranges[0]
        nc.sync.dma_start(xt3[:, :, r0:r1], xT3d[:, :, r0:r1])
        nc.scalar.dma_start(mask_sb[:, :], cst_d.ap())
        for g0, g1 in dma_ranges[1:]:
            nc.sync.dma_start(xt3[:, :, g0:g1], xT3d[:, :, g0:g1])

        def arrive_ns(col):
            cum = 2000.0
            for g0, g1 in dma_ranges:
                cum += 4.63 * (g1 - g0)  # 1536 B/col at ~332 GB/s
                if g0 < col <= g1:
                    break
            return cum

        # ---- projections (bf16) ----
        def kv_group(g0, g1):
            """project [Wk|Wv] for x cols [g0,g1); k rows 0:64, v 64:128."""
            g = g1 - g0
            bal.advance(arrive_ns(g1))
            ps = prpsum.tile([128, 512], F32, tag="proj", name="kvps")
            for c in range(6):
                nc.tensor.matmul(
                    ps[:, 0:g],
                    lhsT=w3[:, c, 0:128],
                    rhs=xt3[:, c, g0:g1],
                    start=(c == 0),
                    stop=(c == 5),
                )
            eng = bal.pick_copy(g)
            if eng == "act":
                nc.scalar.activation(
                    kq_sb[:, g0:g1], ps[:, 0:g],
                    mybir.ActivationFunctionType.Copy,
                )
            else:
                nc.vector.tensor_copy(kq_sb[:, g0:g1], ps[:, 0:g])
            # v token-major via DMA transpose, then interleave ones (DVE 4x)
            t0, t1 = g0 // 128, g1 // 128
            nc.sync.dma_start_transpose(vn3[:, t0:t1, :], kq_sb[64:128, g0:g1])
            nc.vector.tensor_copy(vx3[:, t0:t1, 0:64], vn3[:, t0:t1, :])
            bal.t["dve"] += DVE_COL4 * 64 * (t1 - t0) + DVE_INST

        def q_group(g0, g1):
            """project Wq for x cols [g0,g1) -> kq rows 0:64 at NK+g0-Q0."""
            g = g1 - g0
            bal.advance(arrive_ns(g1))
            ps = prpsum.tile([128, 512], F32, tag="proj", name="qps")
            for c in range(6):
                nc.tensor.matmul(
                    ps[0:64, 0:g],
                    lhsT=w3[:, c, 128:192],
                    rhs=xt3[:, c, g0:g1],
                    start=(c == 0),
                    stop=(c == 5),
                )
            c0 = NK + g0 - Q0
            eng = bal.pick_copy(g)
            if eng == "act":
                nc.scalar.activation(
                    kq_sb[0:64, c0 : c0 + g], ps[0:64, 0:g],
                    mybir.ActivationFunctionType.Copy,
                )
            else:
                nc.vector.tensor_copy(kq_sb[0:64, c0 : c0 + g], ps[0:64, 0:g])

        done = {"kv": 0, "q": Q0}

        def emit_q_upto(tok):
            while done["q"] < min(tok, Q0 + NQ):
                g0 = done["q"]
                g1 = min(g0 + 512, Q0 + NQ)
                q_group(g0, g1)
                done["q"] = g1

        # ---- attention ----
        def emit_s(qc0, Nc, grp):
            """bf16 score matmuls for one group; returns the s psum tile."""
            i0g = max(0, 128 * grp[0] - qc0)
            s_tile = spsum.tile([128, 1024], F32, tag="s", name="s_tile")
            qb = NK + (qc0 - Q0)
            for tl, t in enumerate(grp):
                nc.tensor.matmul(
                    s_tile[:, 512 * tl + i0g : 512 * tl + Nc],
                    lhsT=kq_sb[0:64, 128 * t : 128 * (t + 1)],
                    rhs=kq_sb[0:64, qb + i0g : qb + Nc],
                    start=True,
                    stop=True,
                    skip_group_check=True,
                )
            return s_tile

        def emit_exp(qc0, Nc, grp, s_tile):
            """exp + masks for one group; returns the bf16 P view."""
            ng = len(grp)
            i0g = max(0, 128 * grp[0] - qc0)
            s3 = s_tile.rearrange("p (t i) -> p t i", i=512)
            eng = bal.pick_exp(ng * (Nc - i0g))
            if eng == "act":
                p_tile = ppool.tile([128, 1024], BF16, tag="p", name="p_tile")
                pv = p_tile.rearrange("p (t i) -> p t i", i=512)
                nc.scalar.activation(
                    pv[:, 0:ng, i0g:Nc], s3[:, 0:ng, i0g:Nc],
                    mybir.ActivationFunctionType.Exp,
                    bias=zbias[:, :], scale=SCALE,
                )
            else:
                ti = ipool.tile([128, 1024], I16, tag="ti", name="ti")
                ti3 = ti.rearrange("p (t i) -> p t i", i=512)
                nc.vector.tensor_scalar(
                    ti3[:, 0:ng, i0g:Nc], s3[:, 0:ng, i0g:Nc],
                    C1, CB16, mybir.AluOpType.mult, mybir.AluOpType.add,
                )
                pv = ti.bitcast(BF16).rearrange("p (t i) -> p t i", i=512)
            # exact triangle masks on diagonal tiles
            for tl, t in enumerate(grp):
                dcol = 128 * t - qc0
                if dcol >= 0:
                    blk = pv[:, tl, dcol : dcol + 128]
                    meng = bal.pick_mask()
                    e = nc.gpsimd if meng == "pool" else nc.vector
                    e.tensor_tensor(blk, blk, mask_sb[:, :], op=mybir.AluOpType.mult)
            return pv

        def emit_pv(qc0, Nc, grp, ostate, pv):
            o_tile = ostate["o"]
            for tl, t in enumerate(grp):
                i0 = max(0, 128 * t - qc0)
                start = not ostate["started"]
                ostate["started"] = True
                ostate["left"] -= 1
                stop = ostate["left"] == 0
                nc.tensor.matmul(
                    o_tile[:, i0:Nc],
                    lhsT=vx3[:, t, :],
                    rhs=pv[:, tl, i0:Nc],
                    start=start,
                    stop=stop,
                    skip_group_check=True,
                )

        def finish_chunk(o_tile, ql0, Nc):
            def fin():
                o_sb = fpool.tile([65, 512], F32, tag="osb", name="osb")
                eng = bal.pick_copy(Nc)
                if eng == "act":
                    nc.scalar.activation(
                        o_sb[:, 0:Nc], o_tile[:, 0:Nc],
                        mybir.ActivationFunctionType.Copy,
                    )
                else:
                    nc.vector.tensor_copy(o_sb[:, 0:Nc], o_tile[:, 0:Nc])
                nc.sync.dma_start(oext_d.ap()[:, ql0 : ql0 + Nc], o_sb[:, 0:Nc])

            return fin

        def chunk_groups(qc0, Nc):
            T_c = (qc0 + Nc) // 128
            return T_c, [list(range(t, min(t + 2, T_c))) for t in range(0, T_c, 2)]

        chunks = _chunks_for(Q0, NQ)
        LOOK = 2
        LOOK_PV = 4

        from collections import deque

        class GroupPipe:
            """S emitted LOOK groups before exp (PSUM-limited); PV trails
            its exp by LOOK_PV further groups (p-tile-limited) so the PE
            never waits on a just-issued exp."""

            def __init__(self):
                self.sq = deque()
                self.pq = deque()

            def push(self, qc0, Nc, ql0, grp, ostate, T_c, after=None):
                s = emit_s(qc0, Nc, grp)
                self.sq.append((qc0, Nc, grp, ostate, s, after))
                if len(self.sq) > LOOK:
                    self.pop_exp()

            def pop_exp(self):
                qc0, Nc, grp, ostate, s, after = self.sq.popleft()
                pv = emit_exp(qc0, Nc, grp, s)
                self.pq.append((qc0, Nc, grp, ostate, pv, after))
                if len(self.pq) > LOOK_PV:
                    self.pop_pv()

            def pop_pv(self):
                qc0, Nc, grp, ostate, pv, after = self.pq.popleft()
                emit_pv(qc0, Nc, grp, ostate, pv)
                if after is not None:
                    after()

            def flush(self):
                while self.sq:
                    self.pop_exp()
                while self.pq:
                    self.pop_pv()

        pipe = GroupPipe()

        def make_ostate(groups, name):
            o_tile = opsum.tile([65, 512], F32, tag="ot", name=name)
            # PV matmul count = total tiles in the chunk
            return {
                "o": o_tile,
                "started": False,
                "left": sum(len(g) for g in groups),
            }

        if not b_mode:
            def emit_chunk(ci):
                qc0, Nc = chunks[ci]
                ql0 = qc0 - Q0
                T_c, groups = chunk_groups(qc0, Nc)
                ostate = make_ostate(groups, f"o_tile{ci}")
                fin = finish_chunk(ostate["o"], ql0, Nc)
                for gi, grp in enumerate(groups):
                    after = fin if gi == len(groups) - 1 else None
                    pipe.push(qc0, Nc, ql0, grp, ostate, T_c, after)

            g0 = 0
            ci = 0
            while g0 < NK:
                g1 = min(g0 + 512, NK)
                kv_group(g0, g1)
                done["kv"] = g1
                q_group(g0, g1)
                done["q"] = g1
                g0 = g1
                while ci < len(chunks) and chunks[ci][0] + chunks[ci][1] <= g1:
                    emit_chunk(ci)
                    ci += 1
            while ci < len(chunks):
                emit_chunk(ci)
                ci += 1
            pipe.flush()
        else:
            # B: q cols + late kv tiles first, then stream kv [0, Q0)
            g0 = Q0
            while g0 < NK:
                g1 = min(g0 + 512, NK)
                kv_group(g0, g1)
                g0 = g1
            emit_q_upto(Q0 + NQ)
            qc0, Nc = chunks[0]
            T_c0, groups0 = chunk_groups(qc0, Nc)
            hi_t = Q0 // 128
            early = [g for g in groups0 if g[0] >= hi_t]
            rest = [g for g in groups0 if g[0] < hi_t]
            order0 = early + rest
            ostate0 = make_ostate(groups0, "o_tile0")
            fin0 = finish_chunk(ostate0["o"], 0, Nc)

            def push0(grp):
                after = fin0 if grp is order0[-1] else None
                pipe.push(qc0, Nc, 0, grp, ostate0, T_c0, after)

            for grp in early:
                push0(grp)
            gi = 0
            g0 = 0
            while g0 < Q0:
                g1 = min(g0 + 512, Q0)
                kv_group(g0, g1)
                done["kv"] = g1
                avail = g1 // 128
                while gi < len(rest) and rest[gi][-1] < avail:
                    push0(rest[gi])
                    gi += 1
                g0 = g1
            while gi < len(rest):
                push0(rest[gi])
                gi += 1
            for qc0, Nc in chunks[1:]:
                ql0 = qc0 - Q0
                T_c, groups = chunk_groups(qc0, Nc)
                ostate = make_ostate(groups, "o_tileN")
                fin = finish_chunk(ostate["o"], ql0, Nc)
                for gi2, grp in enumerate(groups):
                    after = fin if gi2 == len(groups) - 1 else None
                    pipe.push(qc0, Nc, ql0, grp, ostate, T_c, after)
            pipe.flush()

    nc.compile()
    return nc


_cache = {}


def _programs():
    if "progs" not in _cache:
        _cache["progs"] = (
            build_half(SPLIT, 0, SPLIT),
            build_half(N, SPLIT, N - SPLIT, b_mode=True),
        )
    return _cache["progs"]


def _host_inputs(x, W_query, W_keys, W_value):
    wqkv = np.concatenate([W_keys, W_value, W_query], axis=1).astype(NPBF16)
    cst = np.triu(np.ones((128, 128), np.float32)).astype(NPBF16)
    xT = np.ascontiguousarray(np.transpose(x, (0, 2, 1))).astype(NPBF16)
    in_A = [
        {"xT": np.ascontiguousarray(xT[b, :, :SPLIT]), "wqkv": wqkv, "cst": cst}
        for b in range(B)
    ]
    in_B = [{"xT": xT[b], "wqkv": wqkv, "cst": cst} for b in range(B)]
    return in_A, in_B


def kernel(x, W_query, W_keys, W_value, _trace=False, _tracedir=None):
    nc_a, nc_b = _programs()
    in_A, in_B = _host_inputs(x, W_query, W_keys, W_value)
    kw = {}
    if _trace:
        kw = dict(trace=True, trace_cores=[0], tmpdir=_tracedir)
    res_a = run_bass_kernel_spmd(nc_a, in_A, core_ids=[0, 1, 2, 3], **kw)
    res_b = run_bass_kernel_spmd(nc_b, in_B, core_ids=[4, 5, 6, 7], **kw)
    out = np.empty((B, N, D_OUT), np.float32)
    for b in range(B):
        oa = res_a.results[b]["oext"]
        ob = res_b.results[b]["oext"]
        out[b, :SPLIT] = (oa[0:64] / oa[64:65]).T
        out[b, SPLIT:] = (ob[0:64] / ob[64:65]).T
    _cache["last_exec_ns"] = (res_a.exec_time_ns, res_b.exec_time_ns)
    return out
